# revision 1
# baseline (speedup 1.0000x reference)
"""Trainium2 Bass kernel for nn_CognitiveNetwork (moe_routing).

Strategy: data-parallel over batch across 8 NeuronCores (each core gets
B/8 = 256 rows; the network is batch-row independent so no collectives).
All activations live feature-on-partition ([h, b] layout, 8 chunks of 128
partitions x 256 batch cols). Weights are host-transposed to W^T and cast
to bf16 so every matmul lhsT slice is a contiguous SBUF view. The routed
cell sequence is known at kernel-build time, so the 24-step program is
fully unrolled and specialized: first occurrence of a cell skips the
W_hh matmuls (h=0) and the whole f-gate (c=0); LSTM state round-trips
through DRAM scratch only when a cell recurs non-adjacently.
"""

import os
import numpy as np
import ml_dtypes

import concourse.bass as bass
import concourse.mybir as mybir
import concourse.tile as tile
from concourse.bass_utils import run_bass_kernel_spmd

BF16NP = np.float16
F32 = mybir.dt.float32
BF = mybir.dt.float16
AF = mybir.ActivationFunctionType
OP = mybir.AluOpType

P = 128
H = 1024
NCH = H // P            # 8 feature chunks
NCORES = 8
LN_EPS = 1e-5
N_CELLS = 16

_wsplit_ctr = [0]


def _split_multi_waits(nc):
    """This container's walrus codegen accepts at most ONE sem wait per
    instruction; TileContext attaches several to drains/ops. Split extras
    into preceding same-engine nops (same semantics: engine streams are
    in-order, so waits on a directly preceding nop gate the instruction)."""
    n_split = 0
    for f in nc.m.functions:
        for bb in f.blocks:
            if not any(
                i.sync_info and i.sync_info.on_wait and len(i.sync_info.on_wait) > 1
                for i in bb.instructions
            ):
                continue
            new = []
            for inst in bb.instructions:
                si = inst.sync_info
                waits = list(si.on_wait) if si and si.on_wait else []
                if len(waits) > 1:
                    n_split += 1
                    for w in waits[:-1]:
                        _wsplit_ctr[0] += 1
                        new.append(mybir.InstNoOp(
                            name=f"I-wsplit-{_wsplit_ctr[0]}",
                            engine=inst.engine, ins=[], outs=[],
                            sync_info=mybir.SyncInfo(on_wait=[w], on_update=[]),
                        ))
                    inst.sync_info = mybir.SyncInfo(
                        on_wait=[waits[-1]], on_update=list(si.on_update or []))
                new.append(inst)
            bb.instructions = new
    return n_split


def _analyze(seq):
    """Occurrence structure of the routed cell sequence."""
    slots = []                      # cells in first-use order
    slot_of = {}
    occs = {}                       # cell -> [step indices]
    for t, e in enumerate(seq):
        if e not in slot_of:
            slot_of[e] = len(slots)
            slots.append(e)
        occs.setdefault(e, []).append(t)
    rep_cells = [e for e in slots if len(occs[e]) > 1]
    rep_slot_of = {e: i for i, e in enumerate(rep_cells)}
    plan = []
    for t, e in enumerate(seq):
        o = occs[e]
        k = o.index(t)
        first = (k == 0)
        prev_adj = (not first) and (o[k - 1] == t - 1)
        load = (not first) and not prev_adj
        store = (k + 1 < len(o)) and (o[k + 1] > t + 1)
        pass_sbuf = (k + 1 < len(o)) and (o[k + 1] == t + 1)
        plan.append(dict(cell=e, slot=slot_of[e],
                         rep_slot=rep_slot_of.get(e), first=first,
                         prev_adj=prev_adj, load=load, store=store,
                         pass_sbuf=pass_sbuf))
    return slots, rep_cells, plan


def _pack_feat_cols(v2d):
    """[n, C*P] per-cell row-major -> [P, n*C] (partition = within-chunk idx,
    col = cell*C + chunk)."""
    n, tot = v2d.shape
    C = tot // P
    return np.ascontiguousarray(
        v2d.reshape(n, C, P).transpose(2, 0, 1).reshape(P, n * C).astype(np.float32))


def _host_pack(inputs, slots, rep_cells):
    """Transpose/cast/pack all weights on the host (done once per call)."""
    W_p = np.asarray(inputs["W_p"], np.float32)
    W_ih = np.asarray(inputs["W_ih"], np.float32)
    W_hh = np.asarray(inputs["W_hh"], np.float32)
    W_a = np.asarray(inputs["W_a"], np.float32)

    def pack_square(W, cells):
        # W[e]: [H(out), H(in)] -> lhsT view [in, out] -> [P, ic, o] -> [n,128,8192]
        out = np.empty((len(cells), P, NCH * H), BF16NP)
        for i, e in enumerate(cells):
            t = W[e].T.reshape(NCH, P, H).transpose(1, 0, 2)   # [P, ic, o]
            out[i] = t.reshape(P, NCH * H).astype(BF16NP)
        return out

    def pack_gates(W, cells):
        # W[e]: [4H, H]. Quarter q covers out-chunks {2q,2q+1} of each gate.
        # -> [n, 4, 128, 8192]; cols = ic*1024 + g*256 + hh*128 + c
        out = np.empty((len(cells), 4, P, NCH * H), BF16NP)
        for i, e in enumerate(cells):
            a = W[e].reshape(4, 4, 2, P, NCH, P)     # [g, q, hh, c, ic, p]
            a = a.transpose(1, 5, 4, 0, 2, 3)        # [q, p, ic, g, hh, c]
            out[i] = a.reshape(4, P, NCH * H).astype(BF16NP)
        return out

    b_ih = np.asarray(inputs["b_ih"], np.float32)
    b_hh = np.asarray(inputs["b_hh"], np.float32)
    bg = b_ih[slots] + b_hh[slots]                   # [n_used, 4H]
    return dict(
        wp=pack_square(W_p, slots),
        wa=pack_square(W_a, slots),
        wih=pack_gates(W_ih, slots),
        whh=pack_gates(W_hh, rep_cells) if rep_cells else None,
        bp=_pack_feat_cols(np.asarray(inputs["b_p"], np.float32)[slots]),
        bg=_pack_feat_cols(bg),
        gam=_pack_feat_cols(np.asarray(inputs["gamma"], np.float32)[slots]),
        bet=_pack_feat_cols(np.asarray(inputs["beta"], np.float32)[slots]),
        ba=_pack_feat_cols(np.asarray(inputs["b_a"], np.float32)[slots]),
    )


def _build(plan, n_used, n_rep, Bl, gate_sig, n_steps, n_emit=None):
    """Emit the Bass program (shared by all 8 cores; per-core x differs)."""
    nc = bass.Bass()
    BW = NCH * Bl                                    # 2048 free cols

    xin_d = nc.dram_tensor("xin", [P, BW], F32, kind="ExternalInput")
    wp_d = nc.dram_tensor("wp", [n_used, P, NCH * H], BF, kind="ExternalInput")
    wa_d = nc.dram_tensor("wa", [n_used, P, NCH * H], BF, kind="ExternalInput")
    wih_d = nc.dram_tensor("wih", [n_used, 4, P, NCH * H], BF, kind="ExternalInput")
    whh_d = (nc.dram_tensor("whh", [n_rep, 4, P, NCH * H], BF, kind="ExternalInput")
             if n_rep else None)
    bp_d = nc.dram_tensor("bp", [P, n_used * NCH], F32, kind="ExternalInput")
    bg_d = nc.dram_tensor("bg", [P, n_used * 4 * NCH], F32, kind="ExternalInput")
    gam_d = nc.dram_tensor("gam", [P, n_used * NCH], F32, kind="ExternalInput")
    bet_d = nc.dram_tensor("bet", [P, n_used * NCH], F32, kind="ExternalInput")
    ba_d = nc.dram_tensor("ba", [P, n_used * NCH], F32, kind="ExternalInput")
    out_d = nc.dram_tensor("out", [P, BW], F32, kind="ExternalOutput")

    if n_emit is None:
        n_emit = n_steps
    with tile.TileContext(nc) as tc:
        with (
            tc.tile_pool(name="const", bufs=1) as constp,
            tc.tile_pool(name="sb", bufs=2) as sb,
            tc.tile_pool(name="wpool", bufs=2) as wpool,
            tc.tile_pool(name="psum", bufs=8, space="PSUM") as psum,
            tc.tile_pool(name="dram", bufs=1, space="DRAM") as dram,
        ):
            # ---- persistent tiles -------------------------------------
            x_sb = constp.tile([P, BW], F32, name="x_sb")
            nc.sync.dma_start(x_sb[:, :], xin_d[:, :])
            bp_sb = constp.tile([P, n_used * NCH], F32, name="bp_sb")
            nc.sync.dma_start(bp_sb[:, :], bp_d[:, :])
            bg_sb = constp.tile([P, n_used * 4 * NCH], F32, name="bg_sb")
            nc.sync.dma_start(bg_sb[:, :], bg_d[:, :])
            gam_sb = constp.tile([P, n_used * NCH], F32, name="gam_sb")
            nc.sync.dma_start(gam_sb[:, :], gam_d[:, :])
            bet_sb = constp.tile([P, n_used * NCH], F32, name="bet_sb")
            nc.sync.dma_start(bet_sb[:, :], bet_d[:, :])
            ba_sb = constp.tile([P, n_used * NCH], F32, name="ba_sb")
            nc.sync.dma_start(ba_sb[:, :], ba_d[:, :])
            ones = constp.tile([P, P], F32, name="ones")
            nc.vector.memset(ones[:, :], 1.0)
            ones16 = constp.tile([1, P], BF, name="ones16")
            nc.vector.memset(ones16[0:1, :], 1.0)
            eps_sb = constp.tile([1, 1], F32, name="eps_sb")
            nc.vector.memset(eps_sb[0:1, 0:1], float(LN_EPS))
            zero_sb = constp.tile([P, 1], F32, name="zero_sb")
            nc.vector.memset(zero_sb[:, 0:1], 0.0)
            v_sb = constp.tile([P, BW], F32, name="v_sb")

            # DRAM scratch for recurring-cell LSTM state
            hst = {}
            cst = {}
            for rs in range(n_rep):
                hst[rs] = dram.tile([P, BW], BF, name=f"hst{rs}", tag=f"hst{rs}")
                cst[rs] = dram.tile([P, BW], F32, name=f"cst{rs}", tag=f"cst{rs}")

            def bias1(base, s, oc):
                return base[:, s * NCH + oc: s * NCH + oc + 1]

            def gbias(s, g, hc):
                c0 = s * 4 * NCH + g * NCH + hc
                return bg_sb[:, c0: c0 + 1]

            # xi for step 0 is just bf16(x)
            xi_t = sb.tile([P, BW], BF, name="xi0", tag="xi")
            nc.vector.tensor_copy(xi_t[:, :], x_sb[:, :])

            sbuf_state = {}   # cell -> (h_tile(bf16), c_tile(f32)) from prev step

            for t in range(n_emit):
                st = plan[t]
                s = st["slot"]
                first = st["first"]
                use_hh = not first

                # ---- weight loads for this step -----------------------
                wp_t = wpool.tile([P, NCH * H], BF, name=f"wp{t}", tag="smallw")
                nc.sync.dma_start(wp_t[:, :], wp_d[s, :, :])

                # ---- LSTM state in ------------------------------------
                if st["load"]:
                    rs = st["rep_slot"]
                    h_t = sb.tile([P, BW], BF, name=f"hin{t}", tag="hload")
                    nc.sync.dma_start(h_t[:, :], hst[rs][:, :])
                    c_t = sb.tile([P, BW], F32, name=f"cin{t}", tag="c")
                    nc.sync.dma_start(c_t[:, :], cst[rs][:, :])
                elif st["prev_adj"]:
                    h_t, c_t = sbuf_state[st["cell"]]
                else:
                    h_t = None
                    c_t = sb.tile([P, BW], F32, name=f"cnew{t}", tag="c")

                # ---- perception matmuls: p = relu(W_p @ xi + b_p) ------
                p_tiles = []
                for pair in range(4):
                    ps = psum.tile([P, 512], F32, name=f"pp{t}_{pair}", tag="mm")
                    for hf in range(2):
                        oc = pair * 2 + hf
                        for ic in range(NCH):
                            nc.tensor.matmul(
                                ps[:, hf * Bl:(hf + 1) * Bl],
                                wp_t[:, ic * H + oc * P: ic * H + (oc + 1) * P],
                                xi_t[:, ic * Bl:(ic + 1) * Bl],
                                start=(hf == 0 and ic == 0),
                                stop=(hf == 1 and ic == NCH - 1))
                    for hf in range(2):
                        oc = pair * 2 + hf
                        pk = sb.tile([P, 2 * Bl], F32, name=f"pk{t}_{oc}",
                                     tag="ppack", bufs=9)
                        nc.scalar.activation(pk[:, 0:Bl], ps[:, hf * Bl:(hf + 1) * Bl],
                                             AF.Relu, bias=bias1(bp_sb, s, oc))
                        nc.vector.tensor_mul(pk[:, Bl:2 * Bl], pk[:, 0:Bl], pk[:, 0:Bl])
                        p_tiles.append(pk)

                # ---- pre-emit quarter-0 W_hh matmuls (independent of LN) ---
                # keeps the in-order PE queue busy during the stats chain
                q0_ps = {}
                q0_wih = None
                if use_hh:
                    wq_hh0 = wpool.tile([P, NCH * H], BF, name=f"whh{t}_0",
                                        tag="bigw", bufs=3)
                    nc.sync.dma_start(wq_hh0[:, :], whh_d[st["rep_slot"], 0, :, :])
                    for g in range(4):
                        ps = psum.tile([P, 512], F32, name=f"g{t}_0_{g}", tag="mm")
                        q0_ps[g] = ps
                        for hf in range(2):
                            col = g * 2 * P + hf * P
                            dst = ps[:, hf * Bl:(hf + 1) * Bl]
                            for ic in range(NCH):
                                nc.tensor.matmul(
                                    dst, wq_hh0[:, ic * H + col: ic * H + col + P],
                                    h_t[:, ic * Bl:(ic + 1) * Bl],
                                    start=(hf == 0 and ic == 0), stop=False)

                # ---- LN stats (partition reduction via ones-matmul) ----
                stat_ps = psum.tile([1, 512], F32, name=f"st{t}", tag="mm")
                for oc in range(NCH):
                    nc.tensor.matmul(stat_ps[0:1, :], ones[:, 0:1],
                                     p_tiles[oc][:, 0:2 * Bl],
                                     start=(oc == 0), stop=(oc == NCH - 1))
                ssrc = sb.tile([1, 512], F32, name=f"ss{t}", tag="stat")
                nc.vector.tensor_scalar_mul(ssrc[0:1, 0:Bl], stat_ps[0:1, 0:Bl],
                                            1.0 / H)
                musq = sb.tile([1, Bl], F32, name=f"mq{t}", tag="musq", bufs=1)
                nc.vector.tensor_mul(musq[0:1, :], ssrc[0:1, 0:Bl], ssrc[0:1, 0:Bl])
                vart = sb.tile([1, Bl], F32, name=f"vr{t}", tag="vart", bufs=1)
                nc.vector.scalar_tensor_tensor(
                    vart[0:1, :], stat_ps[0:1, Bl:2 * Bl], 1.0 / H, musq[0:1, :],
                    op0=OP.mult, op1=OP.subtract)
                stdt = sb.tile([1, Bl], F32, name=f"sd{t}", tag="stdt", bufs=1)
                nc.scalar.activation(stdt[0:1, :], vart[0:1, :], AF.Sqrt,
                                     bias=eps_sb[0:1, 0:1])
                nc.vector.reciprocal(ssrc[0:1, Bl:2 * Bl], stdt[0:1, :])
                # broadcast [mu | rstd] to all partitions: K=1 matmuls with
                # an fp16 hi/lo split of ssrc (exact to ~2^-21, accumulated
                # in fp32 psum; plain fp32 matmul would truncate operands)
                shi = sb.tile([1, 512], BF, name=f"shi{t}", tag="shi", bufs=1)
                nc.vector.tensor_copy(shi[0:1, :], ssrc[0:1, :])
                slo = sb.tile([1, 512], BF, name=f"slo{t}", tag="slo", bufs=1)
                nc.vector.tensor_sub(slo[0:1, :], ssrc[0:1, :], shi[0:1, :])
                bc_ps = psum.tile([P, 512], F32, name=f"bc{t}", tag="mm")
                nc.tensor.matmul(bc_ps[:, :], ones16[0:1, :], shi[0:1, :],
                                 start=True, stop=False)
                nc.tensor.matmul(bc_ps[:, :], ones16[0:1, :], slo[0:1, :],
                                 start=False, stop=True)

                # ---- LN apply -> ln (bf16) ----------------------------
                ln_t = sb.tile([P, BW], BF, name=f"ln{t}", tag="ln")
                for oc in range(NCH):
                    d = sb.tile([P, Bl], F32, name=f"d{t}_{oc}", tag="lnd", bufs=2)
                    nc.vector.tensor_sub(d[:, :], p_tiles[oc][:, 0:Bl],
                                         bc_ps[:, 0:Bl])
                    e = sb.tile([P, Bl], F32, name=f"e{t}_{oc}", tag="lne", bufs=2)
                    nc.vector.tensor_mul(e[:, :], d[:, :], bc_ps[:, Bl:2 * Bl])
                    nc.scalar.activation(ln_t[:, oc * Bl:(oc + 1) * Bl], e[:, :],
                                         AF.Identity, bias=bias1(bet_sb, s, oc),
                                         scale=bias1(gam_sb, s, oc))

                # ---- gates + LSTM pointwise, per quarter ---------------
                hnew_t = sb.tile([P, BW], BF, name=f"hn{t}", tag="hnew")
                for q in range(4):
                    wq_ih = wpool.tile([P, NCH * H], BF, name=f"wih{t}_{q}",
                                       tag="bigw", bufs=3)
                    nc.sync.dma_start(wq_ih[:, :], wih_d[s, q, :, :])
                    if use_hh and q > 0:
                        wq_hh = wpool.tile([P, NCH * H], BF, name=f"whh{t}_{q}",
                                           tag="bigw", bufs=3)
                        nc.sync.dma_start(wq_hh[:, :], whh_d[st["rep_slot"], q, :, :])
                    gates_ps = {}
                    for g in range(4):
                        if first and g == 1:
                            continue        # f-gate unused when c == 0
                        if use_hh and q == 0:
                            ps = q0_ps[g]
                        else:
                            ps = psum.tile([P, 512], F32, name=f"g{t}_{q}_{g}",
                                           tag="mm")
                        gates_ps[g] = ps
                        for hf in range(2):
                            col = g * 2 * P + hf * P
                            dst = ps[:, hf * Bl:(hf + 1) * Bl]
                            if use_hh and q > 0:
                                for ic in range(NCH):
                                    nc.tensor.matmul(
                                        dst, wq_hh[:, ic * H + col: ic * H + col + P],
                                        h_t[:, ic * Bl:(ic + 1) * Bl],
                                        start=(hf == 0 and ic == 0), stop=False)
                            for ic in range(NCH):
                                nc.tensor.matmul(
                                    dst, wq_ih[:, ic * H + col: ic * H + col + P],
                                    ln_t[:, ic * Bl:(ic + 1) * Bl],
                                    start=(not use_hh and hf == 0 and ic == 0),
                                    stop=(hf == 1 and ic == NCH - 1))
                    # pointwise for chunks hc = 2q, 2q+1 (both halves at once)
                    hc0 = 2 * q
                    qs = slice(hc0 * Bl, (hc0 + 2) * Bl)
                    tsi = sb.tile([P, 2 * Bl], F32, name=f"tsi{t}_{q}", tag="tsi")
                    ttg = sb.tile([P, 2 * Bl], F32, name=f"ttg{t}_{q}", tag="ttg")
                    tso = sb.tile([P, 2 * Bl], F32, name=f"tso{t}_{q}", tag="tso")
                    for hf in range(2):
                        hc = hc0 + hf
                        hs = slice(hf * Bl, (hf + 1) * Bl)
                        nc.scalar.activation(tsi[:, hs], gates_ps[0][:, hs],
                                             AF.Sigmoid, bias=gbias(s, 0, hc))
                        nc.scalar.activation(ttg[:, hs], gates_ps[2][:, hs],
                                             AF.Tanh, bias=gbias(s, 2, hc))
                        nc.scalar.activation(tso[:, hs], gates_ps[3][:, hs],
                                             AF.Sigmoid, bias=gbias(s, 3, hc))
                    if use_hh:
                        # tsi <- sig(i)*tanh(g) in place; tsf <- sig(f)*c in place
                        nc.vector.tensor_mul(tsi[:, :], tsi[:, :], ttg[:, :])
                        tsf = sb.tile([P, 2 * Bl], F32, name=f"tsf{t}_{q}", tag="tsf")
                        for hf in range(2):
                            hc = hc0 + hf
                            hs = slice(hf * Bl, (hf + 1) * Bl)
                            nc.scalar.activation(tsf[:, hs], gates_ps[1][:, hs],
                                                 AF.Sigmoid, bias=gbias(s, 1, hc))
                        nc.vector.tensor_mul(tsf[:, :], tsf[:, :], c_t[:, qs])
                        nc.vector.tensor_add(c_t[:, qs], tsf[:, :], tsi[:, :])
                    else:
                        nc.vector.tensor_mul(c_t[:, qs], tsi[:, :], ttg[:, :])
                    ttc = sb.tile([P, 2 * Bl], F32, name=f"ttc{t}_{q}", tag="ttc")
                    nc.scalar.activation(ttc[:, :], c_t[:, qs], AF.Tanh, bias=zero_sb[:, 0:1])
                    nc.vector.tensor_mul(hnew_t[:, qs], tso[:, :], ttc[:, :])

                sbuf_state[st["cell"]] = (hnew_t, c_t)

                # ---- LSTM state out -----------------------------------
                if st["store"]:
                    rs = st["rep_slot"]
                    nc.sync.dma_start(hst[rs][:, :], hnew_t[:, :])
                    nc.sync.dma_start(cst[rs][:, :], c_t[:, :])

                # ---- association: tanh(W_a @ h_new + b_a) -------------
                wa_t = wpool.tile([P, NCH * H], BF, name=f"wa{t}", tag="smallw")
                nc.sync.dma_start(wa_t[:, :], wa_d[s, :, :])
                tnh = sb.tile([P, BW], F32, name=f"tnh{t}", tag="tnh", bufs=1)
                # ctx_t = 0.8^t * v_t ; v_t = v_{t-1} + 0.2*g*0.8^{-t}*tanh_t
                ccoef = float(0.2 * gate_sig[s] * (0.8 ** (-t)))
                acoef = float(0.2 * (0.8 ** t))
                if t + 1 < n_emit:
                    xi_t = sb.tile([P, BW], BF, name=f"xi{t + 1}", tag="xi")
                for pair in range(4):
                    ps = psum.tile([P, 512], F32, name=f"pa{t}_{pair}", tag="mm")
                    for hf in range(2):
                        oc = pair * 2 + hf
                        for ic in range(NCH):
                            nc.tensor.matmul(
                                ps[:, hf * Bl:(hf + 1) * Bl],
                                wa_t[:, ic * H + oc * P: ic * H + (oc + 1) * P],
                                hnew_t[:, ic * Bl:(ic + 1) * Bl],
                                start=(hf == 0 and ic == 0),
                                stop=(hf == 1 and ic == NCH - 1))
                    # fused per-chunk tail: tanh -> v update -> next xi
                    for hf in range(2):
                        oc = pair * 2 + hf
                        cs = slice(oc * Bl, (oc + 1) * Bl)
                        nc.scalar.activation(tnh[:, cs],
                                             ps[:, hf * Bl:(hf + 1) * Bl],
                                             AF.Tanh, bias=bias1(ba_sb, s, oc))
                        if t == 0:
                            nc.vector.tensor_scalar_mul(v_sb[:, cs], tnh[:, cs],
                                                        ccoef)
                        else:
                            nc.vector.scalar_tensor_tensor(
                                v_sb[:, cs], tnh[:, cs], ccoef, v_sb[:, cs],
                                op0=OP.mult, op1=OP.add)
                        if t + 1 < n_emit:
                            nc.vector.scalar_tensor_tensor(
                                xi_t[:, cs], v_sb[:, cs], acoef, x_sb[:, cs],
                                op0=OP.mult, op1=OP.add)

            nc.sync.dma_start(out_d[:, :], v_sb[:, :])

    _split_multi_waits(nc)
    return nc


last_results = None   # BassKernelResults of the most recent run (for test.py)
last_nc = None
last_in_maps = None


def kernel(**inputs):
    n_exec = inputs.pop("_n_exec", None)
    n_steps = int(inputs.pop("_n_steps", 0)) or None
    seq = [int(v) for v in np.asarray(inputs["cell_indices"]).reshape(-1)]
    if n_steps is None:
        n_steps = len(seq)
    seq = seq[:n_steps]

    x = np.asarray(inputs["x"], np.float32)
    B, Hd = x.shape
    assert Hd == H
    Bl = B // NCORES

    slots, rep_cells, plan = _analyze(seq)
    n_used, n_rep = len(slots), len(rep_cells)
    gl = np.asarray(inputs["gate_logit"], np.float64)
    gate_sig = [1.0 / (1.0 + np.exp(-gl[e])) for e in slots]

    packed = _host_pack(inputs, slots, rep_cells)
    nc = _build(plan, n_used, n_rep, Bl, gate_sig, n_steps, n_emit=n_exec)

    # per-core input maps (weights identical, x sliced)
    xT = np.ascontiguousarray(x.T)                       # [H, B]
    shared = dict(
        wp=packed["wp"], wa=packed["wa"], wih=packed["wih"],
        bp=packed["bp"], bg=packed["bg"], gam=packed["gam"],
        bet=packed["bet"], ba=packed["ba"])
    if n_rep:
        shared["whh"] = packed["whh"]
    in_maps = []
    for c in range(NCORES):
        xc = xT[:, c * Bl:(c + 1) * Bl]                  # [H, Bl]
        xc = np.ascontiguousarray(
            xc.reshape(NCH, P, Bl).transpose(1, 0, 2).reshape(P, NCH * Bl))
        m = dict(shared)
        m["xin"] = xc
        in_maps.append(m)

    res = run_bass_kernel_spmd(nc, in_maps, core_ids=list(range(NCORES)),
                               trace=False)
    global last_results, last_nc, last_in_maps
    last_results = res
    last_nc = nc
    last_in_maps = in_maps

    scale = np.float64(0.8) ** (n_steps - 1)
    outs = []
    for c in range(NCORES):
        v = res.results[c]["out"]                        # [P, NCH*Bl]
        v = v.reshape(P, NCH, Bl).transpose(1, 0, 2).reshape(H, Bl)
        outs.append(v)
    full = np.concatenate(outs, axis=1)                  # [H, B]
    return np.ascontiguousarray((full.T.astype(np.float64) * scale).astype(np.float32))



# revision 5
# speedup vs baseline: 78.5108x; 78.5108x over previous
"""Trainium2 Bass kernel for nn_CognitiveNetwork (moe_routing).

Strategy: data-parallel over batch across 8 NeuronCores (each core gets
B/8 = 256 rows; the network is batch-row independent so no collectives).
Activations live feature-on-partition ([h, b] layout, 8 chunks of 128
partitions x 256 batch cols).

v2 over the first working version:
- W_hh matmuls run in fp8e4 with MatmulPerfMode.DoubleRow (2x PE rate,
  half the DMA bytes). Everything else stays fp16: a numpy ablation of
  e4m3 quantization showed hh-only keeps rel err at 2.6e-3 (vs 4e-2 for
  any other matmul family). Scales: ln x8, W_ih x8, W_hh x4, h x16 so
  every gate PSUM carries 64x and one activation scale=1/64 undoes it.
- gamma folded into W_ih and beta/b_* folded into host-computed biases
  (all zero here -> wide [P,512] zero-bias activations).
- LN stats matmuls move fp16 (p, p^2 kept in fp16) instead of fp32
  (4 cycles/row -> 1).
- LSTM state rides DRAM in fp16/fp8 with loads prefetched a step early;
  weight DMAs for step t+1 are issued throughout step t against deeper
  tile pools, so the PE never waits on the FIFO DMA queue (each stall
  also costs ~3us of tensor-engine P-state ramp-down).
- hh matmuls (which don't depend on LayerNorm) are queued before the
  stats->broadcast chain's consumers to keep the PE busy through it.
"""

import numpy as np
import ml_dtypes

import concourse.bass as bass
import concourse.mybir as mybir
import concourse.tile as tile
from concourse.bass_utils import run_bass_kernel_spmd

F16NP = np.float16
F8NP = ml_dtypes.float8_e4m3
F32 = mybir.dt.float32
F16 = mybir.dt.float16
F8 = mybir.dt.float8e4
AF = mybir.ActivationFunctionType
OP = mybir.AluOpType
DR = mybir.MatmulPerfMode.DoubleRow

P = 128
H = 1024
NCH = H // P            # 8 feature chunks
NCORES = 8
LN_EPS = 1e-5
GORD = [0, 2, 3, 1]     # packed gate order (i, g, o, f) from torch (i,f,g,o)
S_LN = 8.0              # ln scale (folded into rstd)
S_IH = 8.0              # W_ih scale -> ih psum x64
S_HH = 4.0              # W_hh fp8 scale
S_H = 16.0              # h fp8 scale  -> hh psum x64
PS_INV = 1.0 / 64.0

_wsplit_ctr = [0]


def _split_multi_waits(nc):
    """This container's walrus codegen accepts at most ONE sem wait per
    instruction; TileContext attaches several to drains/ops. Split extras
    into preceding same-engine nops (same semantics: engine streams are
    in-order, so waits on a directly preceding nop gate the instruction)."""
    n_split = 0
    for f in nc.m.functions:
        for bb in f.blocks:
            if not any(
                i.sync_info and i.sync_info.on_wait and len(i.sync_info.on_wait) > 1
                for i in bb.instructions
            ):
                continue
            new = []
            for inst in bb.instructions:
                si = inst.sync_info
                waits = list(si.on_wait) if si and si.on_wait else []
                if len(waits) > 1:
                    n_split += 1
                    for w in waits[:-1]:
                        _wsplit_ctr[0] += 1
                        new.append(mybir.InstNoOp(
                            name=f"I-wsplit-{_wsplit_ctr[0]}",
                            engine=inst.engine, ins=[], outs=[],
                            sync_info=mybir.SyncInfo(on_wait=[w], on_update=[]),
                        ))
                    inst.sync_info = mybir.SyncInfo(
                        on_wait=[waits[-1]], on_update=list(si.on_update or []))
                new.append(inst)
            bb.instructions = new
    return n_split


def _analyze(seq):
    """Occurrence structure of the routed cell sequence."""
    slots = []                      # cells in first-use order
    slot_of = {}
    occs = {}                       # cell -> [step indices]
    for t, e in enumerate(seq):
        if e not in slot_of:
            slot_of[e] = len(slots)
            slots.append(e)
        occs.setdefault(e, []).append(t)
    rep_cells = [e for e in slots if len(occs[e]) > 1]
    rep_slot_of = {e: i for i, e in enumerate(rep_cells)}
    plan = []
    for t, e in enumerate(seq):
        o = occs[e]
        k = o.index(t)
        first = (k == 0)
        prev_adj = (not first) and (o[k - 1] == t - 1)
        load = (not first) and not prev_adj
        store = (k + 1 < len(o)) and (o[k + 1] > t + 1)
        pass_sbuf = (k + 1 < len(o)) and (o[k + 1] == t + 1)
        plan.append(dict(cell=e, slot=slot_of[e],
                         rep_slot=rep_slot_of.get(e), first=first,
                         prev_adj=prev_adj, load=load, store=store,
                         pass_sbuf=pass_sbuf))
    return slots, rep_cells, plan


def _pack_feat_cols(v2d):
    """[n, C*P] per-cell row-major -> [P, n*C] (partition = within-chunk idx,
    col = cell*C + chunk)."""
    n, tot = v2d.shape
    C = tot // P
    return np.ascontiguousarray(
        v2d.reshape(n, C, P).transpose(2, 0, 1).reshape(P, n * C).astype(np.float32))


def _host_pack(inputs, slots, rep_cells):
    """Transpose/cast/scale/pack all weights on the host (once per call)."""
    W_p = np.asarray(inputs["W_p"], np.float32)
    W_ih = np.asarray(inputs["W_ih"], np.float32)
    W_hh = np.asarray(inputs["W_hh"], np.float32)
    W_a = np.asarray(inputs["W_a"], np.float32)
    gamma = np.asarray(inputs["gamma"], np.float32)
    beta = np.asarray(inputs["beta"], np.float32)

    def pack_square(W, cells):
        # W[e]: [H(out), H(in)] -> lhsT view [in, out] -> [P, ic, o]
        out = np.empty((len(cells), P, NCH * H), F16NP)
        for i, e in enumerate(cells):
            t = W[e].T.reshape(NCH, P, H).transpose(1, 0, 2)   # [P, ic, o]
            out[i] = t.reshape(P, NCH * H).astype(F16NP)
        return out

    def pack_ih(cells):
        # quarter q covers out-chunks {2q,2q+1} of each gate; gate-major
        # packed order GORD so first-occurrence steps DMA only cols [0,6144)
        # (i,g,o). col = gg*2048 + ic*256 + hf*128 + c. Scaled by S_IH, with
        # gamma folded into the input columns.
        out = np.empty((len(cells), 4, P, NCH * H), F16NP)
        for i, e in enumerate(cells):
            w = W_ih[e] * gamma[e][None, :] * S_IH
            a = w.reshape(4, 4, 2, P, NCH, P)        # [g, q, hf, c, ic, p]
            a = a[GORD]
            a = a.transpose(1, 5, 0, 4, 2, 3)        # [q, p, g, ic, hf, c]
            out[i] = a.reshape(4, P, NCH * H).astype(F16NP)
        return out

    def pack_hh(cells):
        # fp8 DoubleRow lhsT: per (q, gg, hf, jpair): [K=128, 2, 128].
        # col = gg*2048 + hf*1024 + j*256 + i*128 + m, scaled by S_HH.
        out = np.empty((len(cells), 4, P, NCH * H), F8NP)
        for i, e in enumerate(cells):
            w = W_hh[e] * S_HH
            b = w.reshape(4, 4, 2, P, 4, 2, P)       # [g, q, hf, m, j, i, k]
            b = b[GORD]
            b = b.transpose(1, 6, 0, 2, 4, 5, 3)     # [q, k, g, hf, j, i, m]
            out[i] = b.reshape(4, P, NCH * H).astype(F8NP)
        return out

    b_ih = np.asarray(inputs["b_ih"], np.float32)
    b_hh = np.asarray(inputs["b_hh"], np.float32)
    # gate bias with beta folded through W_ih; reordered gate-major (GORD)
    bg = np.stack([b_ih[e] + b_hh[e] + W_ih[e] @ beta[e] for e in slots])
    bg = bg.reshape(len(slots), 4, H)[:, GORD].reshape(len(slots), 4 * H)
    bp = np.asarray(inputs["b_p"], np.float32)[slots]
    ba = np.asarray(inputs["b_a"], np.float32)[slots]
    biases_zero = (not bg.any()) and (not bp.any()) and (not ba.any())
    return dict(
        wp=pack_square(W_p, slots),
        wa=pack_square(W_a, slots),
        wih=pack_ih(slots),
        whh=pack_hh(rep_cells) if rep_cells else None,
        bp=_pack_feat_cols(bp),
        bg=_pack_feat_cols(bg),
        ba=_pack_feat_cols(ba),
        biases_zero=biases_zero,
    )


def _build(plan, n_used, n_rep, Bl, gate_sig, n_steps, biases_zero,
           n_emit=None):
    """Emit the Bass program (shared by all 8 cores; per-core x differs)."""
    nc = bass.Bass()
    BW = NCH * Bl                                    # 2048 free cols
    B2 = 2 * Bl

    xin_d = nc.dram_tensor("xin", [P, BW], F32, kind="ExternalInput")
    wp_d = nc.dram_tensor("wp", [n_used, P, NCH * H], F16, kind="ExternalInput")
    wa_d = nc.dram_tensor("wa", [n_used, P, NCH * H], F16, kind="ExternalInput")
    wih_d = nc.dram_tensor("wih", [n_used, 4, P, NCH * H], F16, kind="ExternalInput")
    whh_d = (nc.dram_tensor("whh", [n_rep, 4, P, NCH * H], F8, kind="ExternalInput")
             if n_rep else None)
    bp_d = nc.dram_tensor("bp", [P, n_used * NCH], F32, kind="ExternalInput")
    bg_d = nc.dram_tensor("bg", [P, n_used * 4 * NCH], F32, kind="ExternalInput")
    ba_d = nc.dram_tensor("ba", [P, n_used * NCH], F32, kind="ExternalInput")
    out_d = nc.dram_tensor("out", [P, BW], F32, kind="ExternalOutput")

    if n_emit is None:
        n_emit = n_steps
    with tile.TileContext(nc) as tc:
        with (
            tc.tile_pool(name="const", bufs=1) as constp,
            tc.tile_pool(name="sb", bufs=2) as sb,
            tc.tile_pool(name="wpool", bufs=2) as wpool,
            tc.tile_pool(name="psum", bufs=7, space="PSUM") as psum,
            tc.tile_pool(name="dram", bufs=1, space="DRAM") as dram,
        ):
            # ---- persistent tiles (x first: step 0 blocks on it) ------
            x_sb = constp.tile([P, BW], F32, name="x_sb")
            nc.sync.dma_start(x_sb[:, :], xin_d[:, :])
            bp_sb = constp.tile([P, n_used * NCH], F32, name="bp_sb")
            nc.sync.dma_start(bp_sb[:, :], bp_d[:, :])
            bg_sb = constp.tile([P, n_used * 4 * NCH], F32, name="bg_sb")
            nc.sync.dma_start(bg_sb[:, :], bg_d[:, :])
            ba_sb = constp.tile([P, n_used * NCH], F32, name="ba_sb")
            nc.sync.dma_start(ba_sb[:, :], ba_d[:, :])
            ones16p = constp.tile([P, 1], F16, name="ones16p")
            nc.vector.memset(ones16p[:, 0:1], 1.0)
            ones16 = constp.tile([1, P], F16, name="ones16")
            nc.vector.memset(ones16[0:1, :], 1.0)
            eps64_sb = constp.tile([1, 1], F32, name="eps64_sb")
            nc.vector.memset(eps64_sb[0:1, 0:1], float(LN_EPS / 64.0))
            v_sb = constp.tile([P, BW], F32, name="v_sb")

            # DRAM scratch for recurring-cell LSTM state (h: fp8 x16, c: f16)
            hst = {}
            cst = {}
            for rs in range(n_rep):
                hst[rs] = dram.tile([P, BW], F8, name=f"hst{rs}", tag=f"hst{rs}")
                cst[rs] = dram.tile([P, BW], F16, name=f"cst{rs}", tag=f"cst{rs}")

            def bias1(base, s, oc):
                return base[:, s * NCH + oc: s * NCH + oc + 1]

            def gbias(s, gg, hc):
                c0 = s * 4 * NCH + gg * NCH + hc
                return bg_sb[:, c0: c0 + 1]

            # ---- weight prefetch machinery ----------------------------
            # pend[t] holds the SBUF tiles DMA'd ahead for step t.
            pend = {}

            def fetch_step(t):
                """Allocate + dma_start one piece of step t's inputs.
                Returns a generator-like callable emitting pieces in order."""
                if t >= n_emit:
                    return []
                st = plan[t]
                s = st["slot"]
                d = pend.setdefault(t, {})
                jobs = []

                def jwp():
                    w = wpool.tile([P, NCH * H], F16, name=f"wp{t}", tag="smallw",
                                   bufs=2)
                    nc.sync.dma_start(w[:, :], wp_d[s, :, :])
                    d["wp"] = w

                jobs.append(jwp)

                def mk_jq(q):
                    def jq():
                        ncols = NCH * H if not st["first"] else 3 * 2048
                        w = wpool.tile([P, NCH * H], F16, name=f"wih{t}_{q}",
                                       tag="bigw", bufs=3)
                        nc.sync.dma_start(w[:, 0:ncols], wih_d[s, q, :, 0:ncols])
                        d.setdefault("wih", {})[q] = w
                        if not st["first"]:
                            w8 = wpool.tile([P, NCH * H], F8, name=f"whh{t}_{q}",
                                            tag="w8", bufs=3)
                            nc.sync.dma_start(w8[:, :], whh_d[st["rep_slot"], q, :, :])
                            d.setdefault("whh", {})[q] = w8
                    return jq

                for q in range(4):
                    jobs.append(mk_jq(q))
                    if q == 0 and st["load"]:
                        def jh():
                            h8 = sb.tile([P, BW], F8, name=f"hin{t}", tag="h8load")
                            nc.sync.dma_start(h8[:, :], hst[st["rep_slot"]][:, :])
                            d["h8"] = h8
                        jobs.append(jh)
                    if q == 1 and st["load"]:
                        def jc():
                            ct = sb.tile([P, BW], F16, name=f"cin{t}", tag="c")
                            nc.sync.dma_start(ct[:, :], cst[st["rep_slot"]][:, :])
                            d["c"] = ct
                        jobs.append(jc)

                def jwa():
                    w = wpool.tile([P, NCH * H], F16, name=f"wa{t}", tag="smallw",
                                   bufs=2)
                    nc.sync.dma_start(w[:, :], wa_d[s, :, :])
                    d["wa"] = w

                jobs.append(jwa)
                return jobs

            # preamble: all of step 0's weights, then step 1 trickles in
            for job in fetch_step(0):
                job()

            # xi for step 0 is just f16(x)
            xi_t = sb.tile([P, BW], F16, name="xi0", tag="xi")
            nc.vector.tensor_copy(xi_t[:, :], x_sb[:, :])

            sbuf_state = {}   # cell -> (h16, h8, c) tiles from prev step

            for t in range(n_emit):
                st = plan[t]
                s = st["slot"]
                first = st["first"]
                use_hh = not first
                d = pend[t]
                nxt = fetch_step(t + 1)   # jobs to interleave through step t
                nj = iter(nxt)

                def kick(n=1):
                    for _ in range(n):
                        j = next(nj, None)
                        if j is not None:
                            j()

                # ---- LSTM state in ------------------------------------
                if st["load"]:
                    h8_t = d["h8"]
                    c_t = d["c"]
                elif st["prev_adj"]:
                    _, h8_t, c_t = sbuf_state[st["cell"]]
                else:
                    h8_t = None
                    c_t = sb.tile([P, BW], F16, name=f"cnew{t}", tag="c")

                wp_t = d["wp"]

                # ---- perception: p = relu(W_p @ xi + b_p), p2 = p*p ----
                # p16 layout: col = oc*512 + {0:p,256:p2} + b
                p16 = sb.tile([P, NCH, 2, Bl], F16, name=f"p16_{t}", tag="p16")
                stat_ps = psum.tile([1, B2], F32, name=f"st{t}", tag="st", bufs=1)
                for pair in range(4):
                    ps = psum.tile([P, B2], F32, name=f"pp{t}_{pair}", tag="mm")
                    for hf in range(2):
                        oc = pair * 2 + hf
                        for ic in range(NCH):
                            nc.tensor.matmul(
                                ps[:, hf * Bl:(hf + 1) * Bl],
                                wp_t[:, ic * H + oc * P: ic * H + (oc + 1) * P],
                                xi_t[:, ic * Bl:(ic + 1) * Bl],
                                start=(hf == 0 and ic == 0),
                                stop=(hf == 1 and ic == NCH - 1))
                    pv3 = p16[:, 2 * pair:2 * pair + 2, :, :]
                    if biases_zero:
                        nc.scalar.activation(pv3[:, :, 0, :], ps[:, :], AF.Relu)
                    else:
                        for hf in range(2):
                            nc.scalar.activation(
                                p16[:, 2 * pair + hf, 0, :],
                                ps[:, hf * Bl:(hf + 1) * Bl],
                                AF.Relu, bias=bias1(bp_sb, s, pair * 2 + hf))
                    nc.vector.tensor_mul(pv3[:, :, 1, :], pv3[:, :, 0, :],
                                         pv3[:, :, 0, :])
                    # stats: accumulate [sum p | sum p2] over partitions
                    for hf in range(2):
                        oc = pair * 2 + hf
                        nc.tensor.matmul(stat_ps[0:1, :], ones16p[:, 0:1],
                                         p16[:, oc, :, :],
                                         start=(oc == 0), stop=(oc == NCH - 1))
                    if pair == 0:
                        kick()   # wp(t+1)

                # ---- stats chain -> [mu | 8/std] broadcast (f16) -------
                ssrc = sb.tile([1, B2], F16, name=f"ss{t}", tag="stat")
                nc.vector.tensor_scalar_mul(ssrc[0:1, 0:Bl], stat_ps[0:1, 0:Bl],
                                            1.0 / H)
                musq = sb.tile([1, Bl], F32, name=f"mq{t}", tag="musq", bufs=1)
                nc.vector.tensor_mul(musq[0:1, :], ssrc[0:1, 0:Bl], ssrc[0:1, 0:Bl])
                vart = sb.tile([1, Bl], F32, name=f"vr{t}", tag="vart", bufs=1)
                nc.vector.scalar_tensor_tensor(
                    vart[0:1, :], stat_ps[0:1, Bl:B2], 1.0 / H, musq[0:1, :],
                    op0=OP.mult, op1=OP.subtract)
                stdt = sb.tile([1, Bl], F32, name=f"sd{t}", tag="stdt", bufs=1)
                nc.scalar.activation(stdt[0:1, :], vart[0:1, :], AF.Sqrt,
                                     bias=eps64_sb[0:1, 0:1], scale=PS_INV)

                # ---- hh matmuls for q0 queue BEFORE bc so the PE stays
                # busy while the stats chain finishes on DVE/ACT ----------
                gates_q = {}          # (q) -> {gg: psum}
                used_gg = [0, 1, 2] if first else [0, 1, 2, 3]

                def emit_hh(q):
                    w8 = d["whh"][q]
                    gp = gates_q.setdefault(q, {})
                    for gg in range(4):
                        ps = gp.get(gg)
                        if ps is None:
                            ps = psum.tile([P, B2], F32, name=f"g{t}_{q}_{gg}",
                                           tag="mm")
                            gp[gg] = ps
                        for hf in range(2):
                            base = gg * 2048 + hf * 1024
                            dst = ps[:, hf * Bl:(hf + 1) * Bl]
                            for j in range(4):
                                lw = w8[:, base + j * 256: base + (j + 1) * 256]
                                rh = h8_t[:, 2 * j * Bl:(2 * j + 2) * Bl]
                                nc.tensor.matmul(
                                    dst,
                                    lw.rearrange("p (two m) -> p two m", two=2),
                                    rh.rearrange("p (two n) -> p two n", two=2),
                                    start=(hf == 0 and j == 0), stop=False,
                                    perf_mode=DR)

                if use_hh:
                    emit_hh(0)

                bc_ps = psum.tile([P, B2], F32, name=f"bc{t}", tag="mm")
                nc.tensor.matmul(bc_ps[:, 0:Bl], ones16[0:1, :], ssrc[0:1, 0:Bl],
                                 start=True, stop=False)
                with nc.allow_low_precision("f16 rstd: 5e-4 rel is within budget"):
                    nc.vector.reciprocal(ssrc[0:1, Bl:B2], stdt[0:1, :])
                nc.tensor.matmul(bc_ps[:, Bl:B2], ones16[0:1, :],
                                 ssrc[0:1, Bl:B2], start=False, stop=True)

                # ---- ln16 = 8*(p - mu)*rstd ---------------------------
                ln_t = sb.tile([P, BW], F16, name=f"ln{t}", tag="ln")
                for ic in range(NCH):
                    dd = sb.tile([P, Bl], F16, name=f"d{t}_{ic}", tag="lnd",
                                 bufs=2)
                    nc.vector.tensor_sub(dd[:, :], p16[:, ic, 0, :],
                                         bc_ps[:, 0:Bl])
                    nc.vector.tensor_mul(ln_t[:, ic * Bl:(ic + 1) * Bl],
                                         dd[:, :], bc_ps[:, Bl:B2])

                # ---- gates + LSTM pointwise, per quarter ---------------
                hnew = sb.tile([P, BW], F16, name=f"hn{t}", tag="hnew")
                h8new = (sb.tile([P, BW], F8, name=f"hn8_{t}", tag="h8new")
                         if st["store"] or st["pass_sbuf"] else None)
                for q in range(4):
                    if use_hh and q > 0:
                        emit_hh(q)
                    gp = gates_q.setdefault(q, {})
                    for ic in range(NCH):
                        for gg in used_gg:
                            ps = gp.get(gg)
                            if ps is None:
                                ps = psum.tile([P, B2], F32, name=f"g{t}_{q}_{gg}",
                                               tag="mm")
                                gp[gg] = ps
                            for hf in range(2):
                                col = gg * 2048 + ic * 256 + hf * 128
                                nc.tensor.matmul(
                                    ps[:, hf * Bl:(hf + 1) * Bl],
                                    d["wih"][q][:, col: col + P],
                                    ln_t[:, ic * Bl:(ic + 1) * Bl],
                                    start=(not use_hh and ic == 0 and hf == 0),
                                    stop=(ic == NCH - 1 and hf == 1))
                    kick(2)
                    # pointwise for chunks hc = 2q, 2q+1 (both halves at once)
                    qs = slice(2 * q * Bl, (2 * q + 2) * Bl)
                    tsi = sb.tile([P, B2], F16, name=f"tsi{t}_{q}", tag="tsi")
                    ttg = sb.tile([P, B2], F16, name=f"ttg{t}_{q}", tag="ttg")
                    tso = sb.tile([P, B2], F16, name=f"tso{t}_{q}", tag="tso")
                    if biases_zero:
                        nc.scalar.activation(tsi[:, :], gp[0][:, :], AF.Sigmoid,
                                             scale=PS_INV)
                        nc.scalar.activation(ttg[:, :], gp[1][:, :], AF.Tanh,
                                             scale=PS_INV)
                        nc.scalar.activation(tso[:, :], gp[2][:, :], AF.Sigmoid,
                                             scale=PS_INV)
                    else:
                        for hf in range(2):
                            hc = 2 * q + hf
                            hs = slice(hf * Bl, (hf + 1) * Bl)
                            nc.scalar.activation(tsi[:, hs], gp[0][:, hs],
                                                 AF.Sigmoid, bias=gbias(s, 0, hc),
                                                 scale=PS_INV)
                            nc.scalar.activation(ttg[:, hs], gp[1][:, hs],
                                                 AF.Tanh, bias=gbias(s, 1, hc),
                                                 scale=PS_INV)
                            nc.scalar.activation(tso[:, hs], gp[2][:, hs],
                                                 AF.Sigmoid, bias=gbias(s, 2, hc),
                                                 scale=PS_INV)
                    if use_hh:
                        tsf = sb.tile([P, B2], F16, name=f"tsf{t}_{q}", tag="tsf")
                        if biases_zero:
                            nc.scalar.activation(tsf[:, :], gp[3][:, :],
                                                 AF.Sigmoid, scale=PS_INV)
                        else:
                            for hf in range(2):
                                hc = 2 * q + hf
                                hs = slice(hf * Bl, (hf + 1) * Bl)
                                nc.scalar.activation(tsf[:, hs], gp[3][:, hs],
                                                     AF.Sigmoid,
                                                     bias=gbias(s, 3, hc),
                                                     scale=PS_INV)
                        nc.vector.tensor_mul(tsi[:, :], tsi[:, :], ttg[:, :])
                        nc.vector.tensor_mul(tsf[:, :], tsf[:, :], c_t[:, qs])
                        nc.vector.tensor_add(c_t[:, qs], tsf[:, :], tsi[:, :])
                    else:
                        nc.vector.tensor_mul(c_t[:, qs], tsi[:, :], ttg[:, :])
                    ttc = sb.tile([P, B2], F16, name=f"ttc{t}_{q}", tag="ttc")
                    nc.scalar.activation(ttc[:, :], c_t[:, qs], AF.Tanh)
                    nc.vector.tensor_mul(hnew[:, qs], tso[:, :], ttc[:, :])
                    if h8new is not None:
                        nc.vector.scalar_tensor_tensor(
                            h8new[:, qs], tso[:, :], S_H, ttc[:, :],
                            op0=OP.mult, op1=OP.mult)

                sbuf_state[st["cell"]] = (hnew, h8new, c_t)

                # ---- LSTM state out -----------------------------------
                if st["store"]:
                    rs = st["rep_slot"]
                    nc.sync.dma_start(hst[rs][:, :], h8new[:, :])
                    nc.sync.dma_start(cst[rs][:, :], c_t[:, :])

                # ---- association: tanh(W_a @ h_new + b_a) --------------
                wa_t = d["wa"]
                # ctx_t = 0.8^t * v_t ; v_t = v_{t-1} + 0.2*g*0.8^{-t}*tanh_t
                ccoef = float(0.2 * gate_sig[s] * (0.8 ** (-t)))
                acoef = float(0.2 * (0.8 ** t))
                if t + 1 < n_emit:
                    xi_t = sb.tile([P, BW], F16, name=f"xi{t + 1}", tag="xi")
                for pair in range(4):
                    ps = psum.tile([P, B2], F32, name=f"pa{t}_{pair}", tag="mm")
                    for hf in range(2):
                        oc = pair * 2 + hf
                        for ic in range(NCH):
                            nc.tensor.matmul(
                                ps[:, hf * Bl:(hf + 1) * Bl],
                                wa_t[:, ic * H + oc * P: ic * H + (oc + 1) * P],
                                hnew[:, ic * Bl:(ic + 1) * Bl],
                                start=(hf == 0 and ic == 0),
                                stop=(hf == 1 and ic == NCH - 1))
                    # fused tail: tanh -> v update -> next xi
                    tnh = sb.tile([P, B2], F16, name=f"tnh{t}_{pair}", tag="tnh")
                    if biases_zero:
                        nc.scalar.activation(tnh[:, :], ps[:, :], AF.Tanh)
                    else:
                        for hf in range(2):
                            oc = pair * 2 + hf
                            nc.scalar.activation(tnh[:, hf * Bl:(hf + 1) * Bl],
                                                 ps[:, hf * Bl:(hf + 1) * Bl],
                                                 AF.Tanh, bias=bias1(ba_sb, s, oc))
                    cs = slice(pair * B2, (pair + 1) * B2)
                    if t == 0:
                        nc.vector.tensor_scalar_mul(v_sb[:, cs], tnh[:, :], ccoef)
                    else:
                        nc.vector.scalar_tensor_tensor(
                            v_sb[:, cs], tnh[:, :], ccoef, v_sb[:, cs],
                            op0=OP.mult, op1=OP.add)
                    if t + 1 < n_emit:
                        nc.vector.scalar_tensor_tensor(
                            xi_t[:, cs], v_sb[:, cs], acoef, x_sb[:, cs],
                            op0=OP.mult, op1=OP.add)
                    if pair == 3:
                        kick()    # wa(t+1)

                kick(8)   # flush any remaining prefetch jobs for t+1

            nc.sync.dma_start(out_d[:, :], v_sb[:, :])

    _split_multi_waits(nc)
    return nc


last_results = None   # BassKernelResults of the most recent run (for test.py)
last_nc = None
last_in_maps = None


def kernel(**inputs):
    n_exec = inputs.pop("_n_exec", None)
    n_steps = int(inputs.pop("_n_steps", 0)) or None
    seq = [int(v) for v in np.asarray(inputs["cell_indices"]).reshape(-1)]
    if n_steps is None:
        n_steps = len(seq)
    seq = seq[:n_steps]

    x = np.asarray(inputs["x"], np.float32)
    B, Hd = x.shape
    assert Hd == H
    Bl = B // NCORES

    slots, rep_cells, plan = _analyze(seq)
    n_used, n_rep = len(slots), len(rep_cells)
    gl = np.asarray(inputs["gate_logit"], np.float64)
    gate_sig = [1.0 / (1.0 + np.exp(-gl[e])) for e in slots]

    packed = _host_pack(inputs, slots, rep_cells)
    nc = _build(plan, n_used, n_rep, Bl, gate_sig, n_steps,
                packed["biases_zero"], n_emit=n_exec)

    # per-core input maps (weights identical, x sliced)
    xT = np.ascontiguousarray(x.T)                       # [H, B]
    shared = dict(
        wp=packed["wp"], wa=packed["wa"], wih=packed["wih"],
        bp=packed["bp"], bg=packed["bg"], ba=packed["ba"])
    if n_rep:
        shared["whh"] = packed["whh"]
    in_maps = []
    for c in range(NCORES):
        xc = xT[:, c * Bl:(c + 1) * Bl]                  # [H, Bl]
        xc = np.ascontiguousarray(
            xc.reshape(NCH, P, Bl).transpose(1, 0, 2).reshape(P, NCH * Bl))
        m = dict(shared)
        m["xin"] = xc
        in_maps.append(m)

    res = run_bass_kernel_spmd(nc, in_maps, core_ids=list(range(NCORES)),
                               trace=False)
    global last_results, last_nc, last_in_maps
    last_results = res
    last_nc = nc
    last_in_maps = in_maps

    scale = np.float64(0.8) ** (n_steps - 1)
    outs = []
    for c in range(NCORES):
        v = res.results[c]["out"]                        # [P, NCH*Bl]
        v = v.reshape(P, NCH, Bl).transpose(1, 0, 2).reshape(H, Bl)
        outs.append(v)
    full = np.concatenate(outs, axis=1)                  # [H, B]
    return np.ascontiguousarray((full.T.astype(np.float64) * scale).astype(np.float32))
